# revision 11
# baseline (speedup 1.0000x reference)
"""Trainium2 Bass kernel for nn_ClassChannelAttention.

Computes: out = x * scale[None, :, None, None] where
  scale[c] = sum_k softmax(channel_attention, axis=-1)[k, c]

Sharding: data-parallel over batch B=16 across 8 cores (2 batches/core);
channel_attention (150, 768) replicated to every core. The softmax+class-sum
is tiny and recomputed on each core (no collectives needed).

Precision: the kernel streams x in/out as bf16 (host converts fp32->bf16 on
the way in and upcasts on the way out; the channel scale itself stays fp32
end-to-end on device). This halves HBM traffic per core (50.3 MB -> 25.2 MB)
— the kernel is purely HBM-bandwidth-bound — at ~2.3e-3 rel-l2 cost, far
under the 2e-2 gate.

Per-core layout: x shard viewed as (384, 16384) bf16 — each SBUF partition
row carries FOUR consecutive channel rows (32 KiB contiguous per partition),
in 4 tiles of (96, 16384). The big DMAs move those bytes BITCAST TO INT32:
the SDMA path moves 8 ELEMENTS/cycle/engine regardless of dtype, so
bf16-typed descriptors run at 16B/cyc (13.6 GB/s/engine, measured) while
4-byte-typed ones hit the full 32B/cyc AXI-port rate (27.2 GB/s/engine).
Loads ride the Sync HWDGE ring and stores the Scalar HWDGE ring so HBM reads
and writes stream concurrently. The channel_attention load is issued FIRST
on the Sync ring: rings drain FIFO, so it lands in ~1.3us before the x
loads; on the other ring it would round-robin packet-by-packet against bulk
x loads and not land for ~30us, stalling the whole scale pipeline (measured).

Scale pipeline (all preamble, ~7us): exp on ACT (no max-subtraction — ca is
N(0,1), fp32 exp cannot overflow) with fused row-sum, DVE reciprocal, then
the softmax normalization and class-sum fold into tiny PE matmuls:
psum[4h+m][p, 0] = sum_k e[k, 4*(96h+p)+m] * recip[k]  (lhsT = strided e
view, rhs = recip column). Each of the 8 (h, m) outputs gets its OWN psum
tile: PSUM accumulation groups are bank-granular, so concurrent start/stop
groups must live in distinct banks (column-slicing one psum tile corrupts
the sums — caught by CoreSim). Scales are then copied to SBUF fp32 tiles:
the DVE tensor_scalar per-partition scalar must come from SBUF to keep the
4x_2p packed mode (a PSUM-sourced scalar drops the multiply to 1x on HW,
3.8x slower end-to-end — measured). Quarter m of x-tile i is scaled by
scales[i % 2][:, m] (bf16 data, step-1, 4B-aligned -> 4x_2p, ~1.1us/quarter).
"""

import numpy as np
import ml_dtypes

import concourse.bacc as bacc
import concourse.mybir as mybir
import concourse.tile as tile
from concourse import bass_utils

N_CORES = 8
B, C, H, W = 16, 768, 64, 64
K_CLS = 150
B_SH = B // N_CORES          # 2 batches per core
F = H * W                    # 4096
P = 128
CPP = 4                      # channels packed per partition row (32 KiB bf16)
ROWS4 = B_SH * C // CPP      # 384 rows in the merged view
P_T = 96                     # partitions per tile
N_TILES = ROWS4 // P_T       # 4 tiles of (96, 16384) per core
N_MAPS = C // CPP // P_T     # 2 distinct channel->partition layouts
F4 = CPP * F                 # 16384
X_BUFS = 4                   # SBUF ring depth: all 4 x tiles in flight

_module_cache = {}


def _body(tc, out, x, ca):
    nc = tc.nc
    f32 = mybir.dt.float32
    Exp = mybir.ActivationFunctionType.Exp

    with (
        tc.tile_pool(name="attn", bufs=2) as attn_pool,
        tc.tile_pool(name="small", bufs=1) as small,
        tc.tile_pool(name="psum", bufs=1, space="PSUM") as psum_pool,
        tc.tile_pool(name="xt", bufs=X_BUFS) as xpool,
    ):
        # scales[h][p, m] = sum-softmax over channel 4*(96h+p) + m.
        scales = [
            small.tile([P_T, CPP], f32, name=f"scale{h}", tag=f"scale{h}")
            for h in range(N_MAPS)
        ]
        psums = [
            psum_pool.tile([P_T, 1], f32, name=f"ps{k}", tag=f"ps{k}")
            for k in range(N_MAPS * CPP)
        ]

        i32 = mybir.dt.float32  # bitcast target for DMA views (same bytes)
        xf = (
            x.rearrange("b c h w -> (b c) (h w)")
            .rearrange("(a four) f -> a (four f)", four=CPP)
            .bitcast(i32)
        )
        of = (
            out.rearrange("b c h w -> (b c) (h w)")
            .rearrange("(a four) f -> a (four f)", four=CPP)
            .bitcast(i32)
        )

        # Softmax over channels per class; classes on partitions (128 + 22).
        row_splits = [(0, 128), (128, K_CLS - 128)]
        for idx, (r0, rn) in enumerate(row_splits):
            at = attn_pool.tile([P, C], f32, tag="attn")
            # FIRST on the Sync ring — see module docstring.
            nc.sync.dma_start(out=at[:rn], in_=ca[r0 : r0 + rn])
            e = attn_pool.tile([P, C], f32, tag="e")
            s = attn_pool.tile([P, 1], f32, tag="s")
            # e = exp(at); s = per-class row sum of e (fused accum).
            nc.scalar.activation(out=e[:rn], in_=at[:rn], func=Exp, accum_out=s[:rn])
            r = attn_pool.tile([P, 1], f32, tag="r")
            nc.vector.reciprocal(out=r[:rn], in_=s[:rn])
            # Class-sum of softmax into channel-on-partition layouts via tiny
            # matmuls; rhs = recip folds the softmax normalization in.
            # e viewed as (cls, 192 channel-quads, 4).
            e_r = e.rearrange("k (q m) -> k q m", m=CPP)
            for h in range(N_MAPS):
                for m in range(CPP):
                    nc.tensor.matmul(
                        psums[CPP * h + m],
                        lhsT=e_r[:rn, h * P_T : (h + 1) * P_T, m],
                        rhs=r[:rn],
                        start=(idx == 0),
                        stop=(idx == len(row_splits) - 1),
                    )
        for h in range(N_MAPS):
            for m in range(CPP):
                nc.scalar.copy(
                    out=scales[h][:, m : m + 1], in_=psums[CPP * h + m]
                )

        # Main scaled copy: 4 tiles of (96, 16384) bf16; quarter m of tile i
        # scaled by scales[i % 2][:, m] (DVE 4x_2p, ~1.1us/quarter — ~17us
        # total DVE, fully hidden under the ~60-70us DMA window).
        for i in range(N_TILES):
            sel = scales[i % N_MAPS]
            rows = slice(i * P_T, (i + 1) * P_T)
            xt = xpool.tile([P_T, F4], mybir.dt.bfloat16, name="xt", tag="xt")
            nc.sync.dma_start(out=xt.bitcast(i32), in_=xf[rows])
            for m in range(CPP):
                nc.vector.tensor_scalar_mul(
                    xt[:, m * F : (m + 1) * F],
                    xt[:, m * F : (m + 1) * F],
                    sel[:, m : m + 1],
                )
            nc.scalar.dma_start(out=of[rows], in_=xt.bitcast(i32))


def _get_module():
    if "nc" in _module_cache:
        return _module_cache["nc"]
    nc = bacc.Bacc(
        "TRN2", target_bir_lowering=False, debug=False, enable_asserts=False
    )
    x = nc.dram_tensor(
        "x", (B_SH, C, H, W), mybir.dt.bfloat16, kind="ExternalInput"
    ).ap()
    ca = nc.dram_tensor(
        "channel_attention", (K_CLS, C), mybir.dt.float32, kind="ExternalInput"
    ).ap()
    out = nc.dram_tensor(
        "out", (B_SH, C, H, W), mybir.dt.bfloat16, kind="ExternalOutput"
    ).ap()
    with tile.TileContext(nc) as tc:
        _body(tc, out, x, ca)
    nc.compile()
    _module_cache["nc"] = nc
    return nc


def _run(x, channel_attention, **spmd_kwargs):
    x = np.ascontiguousarray(np.asarray(x, dtype=np.float32))
    ca = np.ascontiguousarray(np.asarray(channel_attention, dtype=np.float32))
    assert x.shape == (B, C, H, W), x.shape
    assert ca.shape == (K_CLS, C), ca.shape
    xb = x.astype(ml_dtypes.bfloat16)
    nc = _get_module()
    in_maps = [
        {"x": xb[i * B_SH : (i + 1) * B_SH], "channel_attention": ca}
        for i in range(N_CORES)
    ]
    res = bass_utils.run_bass_kernel_spmd(
        nc, in_maps, core_ids=list(range(N_CORES)), **spmd_kwargs
    )
    out = np.concatenate([r["out"] for r in res.results], axis=0).astype(np.float32)
    return out, res


def kernel(x, channel_attention):
    out, _ = _run(x, channel_attention)
    return out
